# revision 40
# baseline (speedup 1.0000x reference)
"""Trainium2 Bass kernel for nn_AttentionLayer: softmax(Q K^T / sqrt(d)).

Data-parallel over batch: 8 batch elements -> 8 NeuronCores, weights
replicated, no collectives.

Algebraic restructure (exact, softmax-invariant): with q = x Wq + bq and
k = x Wk + bk,
    q k^T = x (Wq Wk^T) x^T  +  1 (x Wk bq)^T  +  [terms constant along n]
and row-softmax drops any per-row constant, so
    alpha = softmax_n( (t x^T) / sqrt(d) ),   t = x W' + 1 c2^T,
    W' = Wq Wk^T  (512x512),  c2 = Wk bq.
This replaces one of the two [2048x512x512] projections with a single
[512x512x512] matmul (W') — ~6.6us less PE work per core — and removes
the bk load entirely.

Per core:
  xT    = transpose(x)            (PE f32 transposes, DVE evict->bf16)
  WqT/WkT = transpose(Wq/Wk)      (PE f32 transposes, ACT evict->bf16)
  W'    = WqT^T @ WkT             (TensorE bf16, ACT evict->bf16)
  c2    = WkT^T @ bq              (16 tiny N=1 matmuls, f32 PSUM accum)
  tT    = W'-chunks @ xT + c2     (TensorE bf16, bias evict ACT/DVE)
  S     = tT^T @ xT               (TensorE bf16, accumulate over f-tiles)
  E     = exp(S / sqrt(d)) with fused row-sum accumulate (ACT)
  out   = E / rowsum              (DVE per-partition scalar mul -> bf16)

Schedule notes (from NTFF traces): PE matmul throughput is at roofline
when dense (216ns per 512-wide bf16 MM). The input stream is the
startup constraint — ~6MB at the shared-HBM envelope takes ~25us — so
(a) the 2MB of weights loads FIRST, split over the SP/ACT/GpSimd
queues, because the serial W' -> tT chain depends on them, and (b) the
scores loop is split into half-tiles: each m-tile's n-chunks 0-1 touch
only x groups 0-1 and run while groups 2-3 are still streaming in; the
chunk 2-3 halves and the softmax epilogues follow once xg3 lands. This
keeps the PE dense from first data to last matmul. ACT stays exp-only
in the scores phase (output DMAs issue from SP and GpSimd-SWDGE,
normalization on DVE) so the epilogue keeps pace with the PE. The
end-of-kernel semaphore teardown (~12us for the framework-fixed 254
sems) and ~6us engine-init preamble are fixed costs. The DRAM output is
bf16 (halves the ~17MB/core output stream), upconverted to f32 on the
host; rel err vs the fp32 reference is ~4.5e-3.
"""

import os
import sys

sys.path.insert(0, "/opt/trn_rl_repo")

import numpy as np

import concourse.mybir as mybir
import concourse.tile as tile
from concourse import bacc
from concourse.bass_utils import run_bass_kernel_spmd
from concourse.masks import make_identity

B, S, F, D = 8, 2048, 512, 512
P = 128
ST = S // P   # 16 s-tiles
FT = F // P   # 4  f-tiles (contraction for projections / scores)
NCH = 512     # moving-operand / PSUM-bank chunk along the free axis
SC = S // NCH  # 4 chunks of the s axis
NSPLIT = 7    # m-tiles whose chunk-0/1 halves run ahead of xg3

F32 = mybir.dt.float32
BF16 = mybir.dt.bfloat16

# initial warmups sized to end just under Wq's ~13.5us arrival (they start
# ~7.6us now that the operand comes from a DVE memset); overshooting the
# arrival delays trw0 on the in-order PE queue and is measurably worse
WARMUP_MMS = int(os.environ.get("BASS_ATTN_WARMUP", "8"))
OUT_BF16 = os.environ.get("BASS_ATTN_OUT_BF16", "1") == "1"


def _emit(nc, tc, ctx, x_ext, wq_ext, wk_ext, bq_ext, out_ext):
    Act = mybir.ActivationFunctionType

    consts = ctx.enter_context(tc.tile_pool(name="consts", bufs=1))
    persist = ctx.enter_context(tc.tile_pool(name="persist", bufs=1))
    xstage = ctx.enter_context(tc.tile_pool(name="xstage", bufs=4))
    psum = ctx.enter_context(tc.tile_pool(name="psum", bufs=4, space="PSUM"))
    epool = ctx.enter_context(tc.tile_pool(name="epool", bufs=9))
    opool = ctx.enter_context(tc.tile_pool(name="opool", bufs=2))
    spool = ctx.enter_context(tc.tile_pool(name="spool", bufs=4))

    def ps_tile(name):
        # single unified PSUM tag: 4 bufs x [P, 2, 512] f32 = all 8 banks
        return psum.tile([P, 2, NCH], F32, tag="ps", bufs=4, name=name)

    ident = consts.tile([P, P], F32)
    make_identity(nc, ident[:])
    # Warmup operand from a DVE memset: available ~1.8us before gpsimd's
    # identity (gpsimd's engine preamble is the longest), so the PE starts
    # warming the HAM clock gate that much sooner.
    wrm = consts.tile([P, P], F32)
    nc.vector.memset(wrm[:], 0.0)

    def warm(n, name):
        # short (~0.2-0.4us) garbage matmuls: fill a known data-arrival or
        # evict-latency stall seam to keep the HAM clock-gate fed, sized
        # well under the seam so they never delay real work (an overshooting
        # warmup measurably pushes the whole weight chain back)
        wps = ps_tile(name)
        for _ in range(n):
            nc.tensor.matmul(wps[:, 0, :P], wrm[:], wrm[:], start=True, stop=True)

    # --- PE warmup: garbage matmuls while input DMAs land (HAM -> K=8/8)
    if WARMUP_MMS:
        warm(WARMUP_MMS, "warmps")

    from concourse.tile import add_dep_helper

    def gate(first_insts, prev_insts):
        for fi in first_insts:
            for pi in prev_insts:
                add_dep_helper(fi.ins, pi.ins, reason="input DMA phase chain")

    # --- input streaming.  Per-queue concurrency bounds throughput (a
    # single transfer moves ~25GB/s, one queue ~100-250GB/s), and the
    # whole 6MB runs at the shared-HBM envelope either way, so order by
    # NEED: the 2MB of weights first (they gate the serial W' -> tT g0
    # chain), split over all three queues; then the x groups as 64-row
    # half-tiles, rows 0-63 on the SP chain and rows 64-127 on GpSimd.
    wq_st = xstage.tile([P, FT, D], F32, tag="wstage", bufs=2, name="wqst")
    wk_st = xstage.tile([P, FT, D], F32, tag="wstage", bufs=2, name="wkst")

    def wsub(eng, wst, w_ext, ft):
        return eng.dma_start(wst[:, ft, :], w_ext.ap()[ft * P : (ft + 1) * P, :])

    # ACT queue: Wq halves + bq (ungated, from t=0)
    wsub(nc.scalar, wq_st, wq_ext, 0)
    wsub(nc.scalar, wq_st, wq_ext, 1)
    bqf = consts.tile([P, FT], F32)
    nc.scalar.dma_start(bqf[:], bq_ext.ap().rearrange("(dt p) -> p dt", p=P))
    # SP queue: rest of Wq + half of Wk, then the x chain
    sp_prev = [
        wsub(nc.sync, wq_st, wq_ext, 2),
        wsub(nc.sync, wq_st, wq_ext, 3),
        wsub(nc.sync, wk_st, wk_ext, 0),
        wsub(nc.sync, wk_st, wk_ext, 1),
    ]
    # GpSimd (SWDGE) queue: rest of Wk, then the x chain
    gp_prev = [
        wsub(nc.gpsimd, wk_st, wk_ext, 2),
        wsub(nc.gpsimd, wk_st, wk_ext, 3),
    ]

    def load_x_group_half(t, sg, eng, half):
        insts = []
        lo, hi = (0, 64) if half == 0 else (64, P)
        for j in range(4):
            st = sg * 4 + j
            insts.append(
                eng.dma_start(
                    t[lo:hi, j, :], x_ext.ap()[st * P + lo : st * P + hi, :]
                )
            )
        return insts

    # x groups phase-gated per queue so the earlier-needed groups get the
    # queues' full bandwidth
    xgroups = {}
    for sg in range(SC):
        xgroups[sg] = xstage.tile([P, 4, F], F32, tag="xstage", bufs=4, name=f"xg{sg}")
        sp_insts = load_x_group_half(xgroups[sg], sg, nc.sync, 0)
        gp_insts = load_x_group_half(xgroups[sg], sg, nc.gpsimd, 1)
        gate(sp_insts[:1], sp_prev)
        gate(gp_insts[:1], gp_prev)
        sp_prev, gp_prev = sp_insts, gp_insts

    # persistent bf16 operands
    xT = persist.tile([P, FT, S], BF16, name="xT")       # [f(part), ftile, s]
    wT = [persist.tile([P, FT, D], BF16, name=f"wT{w}") for w in range(2)]
    wp = persist.tile([P, FT, D], BF16, name="wp")       # W' [f1(part), f1t, f2]
    tT = persist.tile([P, FT, S], BF16, name="tT")       # [f2(part), f2t, m]
    c2 = consts.tile([P, FT], F32)                       # bias per f2 partition
    bqb = consts.tile([P, FT], BF16)

    def tr_x(sg, cast_act=False):
        # xT[ft][p, s] = x[s, ft*128+p] for this s-group; two f-tiles per
        # 2-bank PSUM tile, one merged [P,2,512] eviction each.  (An
        # f32->bf16 pre-cast + bf16 transposes was tried and is SLOWER
        # end-to-end: the cast adds ~2us latency on each group's critical
        # path, which outweighs the halved PE transpose cost.)
        xts = xgroups[sg]
        for fp in range(2):
            ps = ps_tile(f"tr{sg}{fp}")
            for k in range(2):
                ft = 2 * fp + k
                for j in range(4):
                    nc.tensor.transpose(
                        ps[:, k, j * P : (j + 1) * P],
                        xts[:, j, ft * P : (ft + 1) * P],
                        ident[:],
                    )
            nc.vector.tensor_copy(
                xT[:, 2 * fp : 2 * fp + 2, sg * NCH : (sg + 1) * NCH], ps[:]
            )

    def tr_w(w, wst):
        # wT[w][p, dt, f] = W[f, dt*128+p]
        for dp in range(2):
            ps = ps_tile(f"wtr{w}{dp}")
            for k in range(2):
                dt = 2 * dp + k
                for ft in range(FT):
                    nc.tensor.transpose(
                        ps[:, k, ft * P : (ft + 1) * P],
                        wst[:, ft, dt * P : (dt + 1) * P],
                        ident[:],
                    )
            nc.scalar.activation(wT[w][:, 2 * dp : 2 * dp + 2, :], ps[:], Act.Identity)

    def emit_wprime():
        # W'[f1, f2] = sum_d Wq[f1, d] Wk[f2, d] = WqT^T @ WkT
        for fp in range(2):
            ps = ps_tile(f"wp{fp}")
            for k in range(2):
                f1c = 2 * fp + k
                for dt in range(FT):
                    nc.tensor.matmul(
                        ps[:, k, :],
                        wT[0][:, dt, f1c * P : (f1c + 1) * P],
                        wT[1][:, dt, :],
                        start=(dt == 0),
                        stop=(dt == FT - 1),
                    )
            nc.scalar.activation(wp[:, 2 * fp : 2 * fp + 2, :], ps[:], Act.Identity)
        # c2[f2] = sum_d Wk[f2, d] bq[d]; tiny N=1 matmuls, f32 PSUM accum
        nc.vector.tensor_copy(bqb[:], bqf[:])
        cps = ps_tile("c2ps")
        for f2c in range(FT):
            for dt in range(FT):
                nc.tensor.matmul(
                    cps[:, 0, f2c : f2c + 1],
                    wT[1][:, dt, f2c * P : (f2c + 1) * P],
                    bqb[:, dt : dt + 1],
                    start=(dt == 0),
                    stop=(dt == FT - 1),
                )
        nc.vector.tensor_copy(c2[:], cps[:, 0, :FT])

    def proj_t_pair(mg, fp, evict_act=True):
        # tT[f2, m] = sum_f1 W'[f1, f2] xT[f1, m] + c2[f2] for f2 chunk pair
        ps = ps_tile(f"pj{mg}{fp}")
        for k in range(2):
            f2c = 2 * fp + k
            for f1c in range(FT):
                nc.tensor.matmul(
                    ps[:, k, :],
                    wp[:, f1c, f2c * P : (f2c + 1) * P],
                    xT[:, f1c, mg * NCH : (mg + 1) * NCH],
                    start=(f1c == 0),
                    stop=(f1c == FT - 1),
                )
        for k in range(2):
            f2c = 2 * fp + k
            dst = tT[:, f2c, mg * NCH : (mg + 1) * NCH]
            bias = c2[:, f2c : f2c + 1]
            if evict_act:
                nc.scalar.activation(dst, ps[:, k, :], Act.Identity, bias=bias)
            else:
                nc.vector.tensor_scalar_add(dst, ps[:, k, :], bias)

    inv_sqrt_d = 1.0 / float(np.sqrt(np.float32(D)))
    ets = {}
    asums = {}

    def score_half(mt, h, et, asum):
        # chunks 2h, 2h+1 of m-tile mt: 8 MMs + one fused exp/accumulate
        ps = ps_tile(f"s{mt}_{h}")
        for k in range(2):
            ncn = 2 * h + k
            for f2c in range(FT):
                nc.tensor.matmul(
                    ps[:, k, :],
                    tT[:, f2c, mt * P : (mt + 1) * P],
                    xT[:, f2c, ncn * NCH : (ncn + 1) * NCH],
                    start=(f2c == 0),
                    stop=(f2c == FT - 1),
                )
        nc.scalar.activation(
            et[:, 2 * h : 2 * h + 2, :],
            ps[:],
            Act.Exp,
            scale=inv_sqrt_d,
            accum_out=asum[:, h : h + 1],
        )

    def epilogue(mt, et, asum):
        rsum = spool.tile([P, 1], F32, tag="rsum")
        nc.vector.reduce_sum(rsum[:], asum[:], axis=mybir.AxisListType.X)
        rrec = spool.tile([P, 1], F32, tag="rrec")
        nc.vector.reciprocal(rrec[:], rsum[:])
        ot = opool.tile([P, SC, NCH], BF16 if OUT_BF16 else F32)
        for h in range(2):
            nc.vector.tensor_scalar_mul(
                ot[:, 2 * h : 2 * h + 2, :], et[:, 2 * h : 2 * h + 2, :], rrec[:]
            )
        # ONE merged output DMA per m-tile, SP/GpSimd alternating; ACT
        # stays exp-only so the epilogue keeps pace with the PE
        dma_eng = nc.sync if mt % 2 == 0 else nc.gpsimd
        dma_eng.dma_start(out_ext.ap()[mt * P : (mt + 1) * P, :], ot[:])

    def new_et_asum(mt):
        ets[mt] = epool.tile([P, SC, NCH], F32, tag="et", bufs=9, name="et")
        asums[mt] = spool.tile([P, 2], F32, tag="asum", bufs=9, name="asum")

    # --- pre-scores: weight-derived chain first (weights land first);
    # short warmup fills in the measured stall seams (Wk arrival ~3.4us
    # after Wq; ACT evict latency between transpose chain stages)
    tr_w(0, wq_st)
    warm(6, "warmw1")
    tr_w(1, wk_st)
    warm(2, "warmw2")
    emit_wprime()
    warm(2, "warmw3")
    tr_x(0)
    proj_t_pair(0, 0)
    proj_t_pair(0, 1)
    tr_x(1)

    # --- P1: chunk-0/1 halves of the first NSPLIT m-tiles (need only
    # x groups 0-1) run while x groups 2-3 are still streaming in;
    # deferred tT projections for m-groups 1-2 interleave (their moving
    # operand is x group 1 / 2 respectively).
    for mt in range(NSPLIT):
        new_et_asum(mt)
        score_half(mt, 0, ets[mt], asums[mt])
        if mt == 0:
            proj_t_pair(1, 0, evict_act=False)
        elif mt == 1:
            proj_t_pair(1, 1, evict_act=False)
        elif mt == 3:
            tr_x(2, cast_act=True)
        elif mt == 4:
            proj_t_pair(2, 0, evict_act=False)
        elif mt == 5:
            proj_t_pair(2, 1, evict_act=False)
    tr_x(3, cast_act=True)

    # --- P2: finish the split m-tiles (chunks 2-3 + epilogue), then the
    # remaining m-tiles in full; tT group 3 interleaves early in P2.
    for mt in range(NSPLIT):
        score_half(mt, 1, ets[mt], asums[mt])
        if mt == 0:
            # evicts stay on DVE: ACT's exps gate PSUM release here, and
            # loading ACT further measurably stalls the PE
            proj_t_pair(3, 0, evict_act=False)
        elif mt == 1:
            proj_t_pair(3, 1, evict_act=False)
        epilogue(mt, ets[mt], asums[mt])
    for mt in range(NSPLIT, ST):
        last_mt = mt == ST - 1
        if not last_mt:
            new_et_asum(mt)
            score_half(mt, 0, ets[mt], asums[mt])
            score_half(mt, 1, ets[mt], asums[mt])
            epilogue(mt, ets[mt], asums[mt])
        else:
            # last m-tile: fine-grained drain — 512-wide exp/normalize
            # chunks, DMAs alternating GpSimd/SP (SP last)
            et = epool.tile([P, SC, NCH], F32, tag="et", bufs=9)
            asum = spool.tile([P, SC + 1], F32, tag="asum", bufs=9)
            pss = [ps_tile(f"sl{i}") for i in range(2)]
            H = NCH // 2
            for ncn in range(SC):
                ps = pss[ncn // 2][:, ncn % 2, :]
                if ncn < SC - 1:
                    for f2c in range(FT):
                        nc.tensor.matmul(
                            ps,
                            tT[:, f2c, mt * P : (mt + 1) * P],
                            xT[:, f2c, ncn * NCH : (ncn + 1) * NCH],
                            start=(f2c == 0),
                            stop=(f2c == FT - 1),
                        )
                    nc.scalar.activation(
                        et[:, ncn, :],
                        ps,
                        Act.Exp,
                        scale=inv_sqrt_d,
                        accum_out=asum[:, ncn : ncn + 1],
                    )
                else:
                    # very last chunk in two 256-wide pieces: the first
                    # piece's exp overlaps the second piece's matmuls,
                    # shortening the post-last-matmul drain chain
                    for h2 in range(2):
                        for f2c in range(FT):
                            nc.tensor.matmul(
                                ps[:, h2 * H : (h2 + 1) * H],
                                tT[:, f2c, mt * P : (mt + 1) * P],
                                xT[:, f2c, ncn * NCH + h2 * H : ncn * NCH + (h2 + 1) * H],
                                start=(f2c == 0),
                                stop=(f2c == FT - 1),
                            )
                        nc.scalar.activation(
                            et[:, ncn, h2 * H : (h2 + 1) * H],
                            ps[:, h2 * H : (h2 + 1) * H],
                            Act.Exp,
                            scale=inv_sqrt_d,
                            accum_out=asum[:, ncn + h2 : ncn + h2 + 1],
                        )
            rsum = spool.tile([P, 1], F32, tag="rsum")
            nc.vector.reduce_sum(rsum[:], asum[:], axis=mybir.AxisListType.X)
            rrec = spool.tile([P, 1], F32, tag="rrec")
            nc.vector.reciprocal(rrec[:], rsum[:])
            ot = opool.tile([P, SC, NCH], BF16 if OUT_BF16 else F32)
            for q in range(SC - 1):
                sl = slice(q * NCH, (q + 1) * NCH)
                if q % 2 == 0:
                    nc.vector.tensor_scalar_mul(ot[:, q, :], et[:, q, :], rrec[:])
                else:
                    nc.scalar.activation(ot[:, q, :], et[:, q, :], Act.Identity, scale=rrec[:])
                dma_eng = nc.gpsimd if q % 2 == 0 else nc.sync
                dma_eng.dma_start(out_ext.ap()[mt * P : (mt + 1) * P, sl], ot[:, q, :])
            q = SC - 1
            for h2 in range(2):
                sl2 = slice(h2 * H, (h2 + 1) * H)
                if h2 == 0:
                    nc.vector.tensor_scalar_mul(ot[:, q, sl2], et[:, q, sl2], rrec[:])
                else:
                    nc.scalar.activation(ot[:, q, sl2], et[:, q, sl2], Act.Identity, scale=rrec[:])
                dma_eng = nc.gpsimd if h2 == 0 else nc.sync
                dma_eng.dma_start(
                    out_ext.ap()[mt * P : (mt + 1) * P, q * NCH + h2 * H : q * NCH + (h2 + 1) * H],
                    ot[:, q, sl2],
                )


_CACHE = {}


def build():
    if "nc" in _CACHE:
        return _CACHE["nc"]
    from contextlib import ExitStack

    nc = bacc.Bacc("TRN2", target_bir_lowering=False, debug=False, num_devices=B)
    x_ext = nc.dram_tensor("x", [S, F], F32, kind="ExternalInput")
    wq_ext = nc.dram_tensor("Wq", [F, D], F32, kind="ExternalInput")
    wk_ext = nc.dram_tensor("Wk", [F, D], F32, kind="ExternalInput")
    bq_ext = nc.dram_tensor("bq", [D], F32, kind="ExternalInput")
    out_ext = nc.dram_tensor(
        "out", [S, S], BF16 if OUT_BF16 else F32, kind="ExternalOutput"
    )

    with tile.TileContext(nc) as tc:
        with ExitStack() as ctx:
            _emit(nc, tc, ctx, x_ext, wq_ext, wk_ext, bq_ext, out_ext)

    nc.compile()
    _CACHE["nc"] = nc
    return nc


def make_in_maps(x, Wq, bq, Wk):
    x = np.ascontiguousarray(np.asarray(x, dtype=np.float32))
    Wq = np.ascontiguousarray(np.asarray(Wq, dtype=np.float32))
    Wk = np.ascontiguousarray(np.asarray(Wk, dtype=np.float32))
    bq = np.ascontiguousarray(np.asarray(bq, dtype=np.float32))
    return [{"x": x[i], "Wq": Wq, "Wk": Wk, "bq": bq} for i in range(B)]


def kernel(x, Wq, bq, Wk, bk=None, Wv=None, bv=None, **_unused):
    nc = build()
    in_maps = make_in_maps(x, Wq, bq, Wk)
    res = run_bass_kernel_spmd(nc, in_maps, core_ids=list(range(B)))
    return np.stack(
        [np.asarray(res.results[i]["out"], dtype=np.float32) for i in range(B)], axis=0
    )


# revision 42
# speedup vs baseline: 1.1472x; 1.1472x over previous
"""Trainium2 Bass kernel for nn_AttentionLayer: softmax(Q K^T / sqrt(d)).

Data-parallel over batch: 8 batch elements -> 8 NeuronCores, weights
replicated, no collectives.

Algebraic restructure (exact, softmax-invariant): with q = x Wq + bq and
k = x Wk + bk,
    q k^T = x (Wq Wk^T) x^T  +  1 (x Wk bq)^T  +  [terms constant along n]
and row-softmax drops any per-row constant, so
    alpha = softmax_n( (t x^T) / sqrt(d) ),   t = x W' + 1 c2^T,
    W' = Wq Wk^T  (512x512),  c2 = Wk bq.
This replaces one of the two [2048x512x512] projections with a single
[512x512x512] matmul (W') — ~6.6us less PE work per core — and removes
the bk load entirely.

Per core:
  xT    = transpose(x)            (PE f32 transposes, DVE evict->bf16)
  WqT/WkT = transpose(Wq/Wk)      (PE f32 transposes, ACT evict->bf16)
  W'    = WqT^T @ WkT             (TensorE bf16, ACT evict->bf16)
  c2    = WkT^T @ bq              (16 tiny N=1 matmuls, f32 PSUM accum)
  tT    = W'-chunks @ xT + c2     (TensorE bf16, bias evict ACT/DVE)
  S     = tT^T @ xT               (TensorE bf16, accumulate over f-tiles)
  E     = exp(S / sqrt(d)) with fused row-sum accumulate (ACT)
  out   = E / rowsum              (DVE per-partition scalar mul -> bf16)

Schedule notes (from NTFF traces): PE matmul throughput is at roofline
when dense (216ns per 512-wide bf16 MM). The input stream is the
startup constraint — ~6MB at the shared-HBM envelope takes ~25us — so
(a) the 2MB of weights loads FIRST, split over the SP/ACT/GpSimd
queues, because the serial W' -> tT chain depends on them, and (b) the
scores loop is split into half-tiles: each m-tile's n-chunks 0-1 touch
only x groups 0-1 and run while groups 2-3 are still streaming in; the
chunk 2-3 halves and the softmax epilogues follow once xg3 lands. This
keeps the PE dense from first data to last matmul. ACT stays exp-only
in the scores phase (output DMAs issue from SP and GpSimd-SWDGE,
normalization on DVE) so the epilogue keeps pace with the PE. The
end-of-kernel semaphore teardown (~12us for the framework-fixed 254
sems) and ~6us engine-init preamble are fixed costs. The DRAM output is
bf16 (halves the ~17MB/core output stream), upconverted to f32 on the
host; rel err vs the fp32 reference is ~4.5e-3.
"""

import os
import sys

sys.path.insert(0, "/opt/trn_rl_repo")

import numpy as np

import concourse.mybir as mybir
import concourse.tile as tile
from concourse import bacc
from concourse.bass_utils import run_bass_kernel_spmd
from concourse.masks import make_identity

B, S, F, D = 8, 2048, 512, 512
P = 128
ST = S // P   # 16 s-tiles
FT = F // P   # 4  f-tiles (contraction for projections / scores)
NCH = 512     # moving-operand / PSUM-bank chunk along the free axis
SC = S // NCH  # 4 chunks of the s axis
NSPLIT = 7    # m-tiles whose chunk-0/1 halves run ahead of xg3

F32 = mybir.dt.float32
BF16 = mybir.dt.bfloat16

# initial warmups sized to end just under Wq's ~13.5us arrival (they start
# ~7.6us now that the operand comes from a DVE memset); overshooting the
# arrival delays trw0 on the in-order PE queue and is measurably worse
WARMUP_MMS = int(os.environ.get("BASS_ATTN_WARMUP", "8"))
OUT_BF16 = os.environ.get("BASS_ATTN_OUT_BF16", "1") == "1"


def _emit(nc, tc, ctx, x_ext, wq_ext, wk_ext, bq_ext, out_ext):
    Act = mybir.ActivationFunctionType

    consts = ctx.enter_context(tc.tile_pool(name="consts", bufs=1))
    persist = ctx.enter_context(tc.tile_pool(name="persist", bufs=1))
    xstage = ctx.enter_context(tc.tile_pool(name="xstage", bufs=4))
    psum = ctx.enter_context(tc.tile_pool(name="psum", bufs=4, space="PSUM"))
    epool = ctx.enter_context(tc.tile_pool(name="epool", bufs=9))
    opool = ctx.enter_context(tc.tile_pool(name="opool", bufs=2))
    spool = ctx.enter_context(tc.tile_pool(name="spool", bufs=4))

    def ps_tile(name):
        # single unified PSUM tag: 4 bufs x [P, 2, 512] f32 = all 8 banks
        return psum.tile([P, 2, NCH], F32, tag="ps", bufs=4, name=name)

    ident = consts.tile([P, P], F32)
    make_identity(nc, ident[:])
    # Warmup operand from a DVE memset: available ~1.8us before gpsimd's
    # identity (gpsimd's engine preamble is the longest), so the PE starts
    # warming the HAM clock gate that much sooner.
    wrm = consts.tile([P, P], F32)
    nc.vector.memset(wrm[:], 0.0)

    def warm(n, name):
        # short (~0.2-0.4us) garbage matmuls: fill a known data-arrival or
        # evict-latency stall seam to keep the HAM clock-gate fed, sized
        # well under the seam so they never delay real work (an overshooting
        # warmup measurably pushes the whole weight chain back)
        wps = ps_tile(name)
        for _ in range(n):
            nc.tensor.matmul(wps[:, 0, :P], wrm[:], wrm[:], start=True, stop=True)

    # --- PE warmup: garbage matmuls while input DMAs land (HAM -> K=8/8)
    if WARMUP_MMS:
        warm(WARMUP_MMS, "warmps")

    from concourse.tile import add_dep_helper

    def gate(first_insts, prev_insts):
        for fi in first_insts:
            for pi in prev_insts:
                add_dep_helper(fi.ins, pi.ins, reason="input DMA phase chain")

    # --- input streaming.  Per-queue concurrency bounds throughput (a
    # single transfer moves ~25GB/s, one queue ~100-250GB/s), and the
    # whole 6MB runs at the shared-HBM envelope either way, so order by
    # NEED: the 2MB of weights first (they gate the serial W' -> tT g0
    # chain), split over all three queues; then the x groups as 64-row
    # half-tiles, rows 0-63 on the SP chain and rows 64-127 on GpSimd.
    wq_st = xstage.tile([P, FT, D], F32, tag="wstage", bufs=2, name="wqst")
    wk_st = xstage.tile([P, FT, D], F32, tag="wstage", bufs=2, name="wkst")

    def wsub(eng, wst, w_ext, ft):
        return eng.dma_start(wst[:, ft, :], w_ext.ap()[ft * P : (ft + 1) * P, :])

    # ACT queue: Wq halves + bq (ungated, from t=0)
    wsub(nc.scalar, wq_st, wq_ext, 0)
    wsub(nc.scalar, wq_st, wq_ext, 1)
    bqf = consts.tile([P, FT], F32)
    nc.scalar.dma_start(bqf[:], bq_ext.ap().rearrange("(dt p) -> p dt", p=P))
    # SP queue: rest of Wq + half of Wk, then the x chain
    sp_prev = [
        wsub(nc.sync, wq_st, wq_ext, 2),
        wsub(nc.sync, wq_st, wq_ext, 3),
        wsub(nc.sync, wk_st, wk_ext, 0),
        wsub(nc.sync, wk_st, wk_ext, 1),
    ]
    # GpSimd (SWDGE) queue: rest of Wk, then the x chain
    gp_prev = [
        wsub(nc.gpsimd, wk_st, wk_ext, 2),
        wsub(nc.gpsimd, wk_st, wk_ext, 3),
    ]

    def load_x_group_half(t, sg, eng, half):
        insts = []
        lo, hi = (0, 64) if half == 0 else (64, P)
        for j in range(4):
            st = sg * 4 + j
            insts.append(
                eng.dma_start(
                    t[lo:hi, j, :], x_ext.ap()[st * P + lo : st * P + hi, :]
                )
            )
        return insts

    # x groups phase-gated per queue so the earlier-needed groups get the
    # queues' full bandwidth
    xgroups = {}
    for sg in range(SC):
        xgroups[sg] = xstage.tile([P, 4, F], F32, tag="xstage", bufs=4, name=f"xg{sg}")
        sp_insts = load_x_group_half(xgroups[sg], sg, nc.sync, 0)
        gp_insts = load_x_group_half(xgroups[sg], sg, nc.gpsimd, 1)
        gate(sp_insts[:1], sp_prev)
        gate(gp_insts[:1], gp_prev)
        sp_prev, gp_prev = sp_insts, gp_insts

    # persistent bf16 operands
    xT = persist.tile([P, FT, S], BF16, name="xT")       # [f(part), ftile, s]
    wT = [persist.tile([P, FT, D], BF16, name=f"wT{w}") for w in range(2)]
    wp = persist.tile([P, FT, D], BF16, name="wp")       # W' [f1(part), f1t, f2]
    tT = persist.tile([P, FT, S], BF16, name="tT")       # [f2(part), f2t, m]
    c2 = consts.tile([P, FT], F32)                       # bias per f2 partition
    bqb = consts.tile([P, FT], BF16)

    def tr_x(sg, cast_act=False):
        # xT[ft][p, s] = x[s, ft*128+p] for this s-group; two f-tiles per
        # 2-bank PSUM tile, one merged [P,2,512] eviction each.  (An
        # f32->bf16 pre-cast + bf16 transposes was tried and is SLOWER
        # end-to-end: the cast adds ~2us latency on each group's critical
        # path, which outweighs the halved PE transpose cost.)
        xts = xgroups[sg]
        for fp in range(2):
            ps = ps_tile(f"tr{sg}{fp}")
            for k in range(2):
                ft = 2 * fp + k
                for j in range(4):
                    nc.tensor.transpose(
                        ps[:, k, j * P : (j + 1) * P],
                        xts[:, j, ft * P : (ft + 1) * P],
                        ident[:],
                    )
            nc.vector.tensor_copy(
                xT[:, 2 * fp : 2 * fp + 2, sg * NCH : (sg + 1) * NCH], ps[:]
            )

    def tr_w(w, wst):
        # wT[w][p, dt, f] = W[f, dt*128+p]
        for dp in range(2):
            ps = ps_tile(f"wtr{w}{dp}")
            for k in range(2):
                dt = 2 * dp + k
                for ft in range(FT):
                    nc.tensor.transpose(
                        ps[:, k, ft * P : (ft + 1) * P],
                        wst[:, ft, dt * P : (dt + 1) * P],
                        ident[:],
                    )
            nc.scalar.activation(wT[w][:, 2 * dp : 2 * dp + 2, :], ps[:], Act.Identity)

    def emit_wprime():
        # W'[f1, f2] = sum_d Wq[f1, d] Wk[f2, d] = WqT^T @ WkT
        for fp in range(2):
            ps = ps_tile(f"wp{fp}")
            for k in range(2):
                f1c = 2 * fp + k
                for dt in range(FT):
                    nc.tensor.matmul(
                        ps[:, k, :],
                        wT[0][:, dt, f1c * P : (f1c + 1) * P],
                        wT[1][:, dt, :],
                        start=(dt == 0),
                        stop=(dt == FT - 1),
                    )
            nc.scalar.activation(wp[:, 2 * fp : 2 * fp + 2, :], ps[:], Act.Identity)
        # c2[f2] = sum_d Wk[f2, d] bq[d]; tiny N=1 matmuls, f32 PSUM accum
        nc.vector.tensor_copy(bqb[:], bqf[:])
        cps = ps_tile("c2ps")
        for f2c in range(FT):
            for dt in range(FT):
                nc.tensor.matmul(
                    cps[:, 0, f2c : f2c + 1],
                    wT[1][:, dt, f2c * P : (f2c + 1) * P],
                    bqb[:, dt : dt + 1],
                    start=(dt == 0),
                    stop=(dt == FT - 1),
                )
        nc.vector.tensor_copy(c2[:], cps[:, 0, :FT])

    def proj_t_pair(mg, fp, evict_act=True):
        # tT[f2, m] = sum_f1 W'[f1, f2] xT[f1, m] + c2[f2] for f2 chunk pair
        ps = ps_tile(f"pj{mg}{fp}")
        for k in range(2):
            f2c = 2 * fp + k
            for f1c in range(FT):
                nc.tensor.matmul(
                    ps[:, k, :],
                    wp[:, f1c, f2c * P : (f2c + 1) * P],
                    xT[:, f1c, mg * NCH : (mg + 1) * NCH],
                    start=(f1c == 0),
                    stop=(f1c == FT - 1),
                )
        for k in range(2):
            f2c = 2 * fp + k
            dst = tT[:, f2c, mg * NCH : (mg + 1) * NCH]
            bias = c2[:, f2c : f2c + 1]
            if evict_act:
                nc.scalar.activation(dst, ps[:, k, :], Act.Identity, bias=bias)
            else:
                nc.vector.tensor_scalar_add(dst, ps[:, k, :], bias)

    inv_sqrt_d = 1.0 / float(np.sqrt(np.float32(D)))
    ets = {}
    asums = {}

    def score_half(mt, h, et, asum):
        # chunks 2h, 2h+1 of m-tile mt: 8 MMs + one fused exp/accumulate
        ps = ps_tile(f"s{mt}_{h}")
        for k in range(2):
            ncn = 2 * h + k
            for f2c in range(FT):
                nc.tensor.matmul(
                    ps[:, k, :],
                    tT[:, f2c, mt * P : (mt + 1) * P],
                    xT[:, f2c, ncn * NCH : (ncn + 1) * NCH],
                    start=(f2c == 0),
                    stop=(f2c == FT - 1),
                )
        nc.scalar.activation(
            et[:, 2 * h : 2 * h + 2, :],
            ps[:],
            Act.Exp,
            scale=inv_sqrt_d,
            accum_out=asum[:, h : h + 1],
        )

    def epilogue(mt, et, asum):
        rsum = spool.tile([P, 1], F32, tag="rsum")
        nc.vector.reduce_sum(rsum[:], asum[:], axis=mybir.AxisListType.X)
        rrec = spool.tile([P, 1], F32, tag="rrec")
        nc.vector.reciprocal(rrec[:], rsum[:])
        ot = opool.tile([P, SC, NCH], BF16 if OUT_BF16 else F32)
        for h in range(2):
            nc.vector.tensor_scalar_mul(
                ot[:, 2 * h : 2 * h + 2, :], et[:, 2 * h : 2 * h + 2, :], rrec[:]
            )
        # ONE merged output DMA per m-tile, SP/GpSimd alternating; ACT
        # stays exp-only so the epilogue keeps pace with the PE
        dma_eng = nc.sync if mt % 2 == 0 else nc.gpsimd
        dma_eng.dma_start(out_ext.ap()[mt * P : (mt + 1) * P, :], ot[:])

    def new_et_asum(mt):
        ets[mt] = epool.tile([P, SC, NCH], F32, tag="et", bufs=9, name="et")
        asums[mt] = spool.tile([P, 2], F32, tag="asum", bufs=9, name="asum")

    # --- pre-scores: weight-derived chain first (weights land first);
    # short warmup fills in the measured stall seams (Wk arrival ~3.4us
    # after Wq; ACT evict latency between transpose chain stages)
    tr_w(0, wq_st)
    warm(6, "warmw1")
    tr_w(1, wk_st)
    warm(2, "warmw2")
    emit_wprime()
    warm(2, "warmw3")
    tr_x(0)
    proj_t_pair(0, 0)
    proj_t_pair(0, 1)
    tr_x(1)

    # --- P1: chunk-0/1 halves of the first NSPLIT m-tiles (need only
    # x groups 0-1) run while x groups 2-3 are still streaming in;
    # deferred tT projections for m-groups 1-2 interleave (their moving
    # operand is x group 1 / 2 respectively).
    for mt in range(NSPLIT):
        new_et_asum(mt)
        score_half(mt, 0, ets[mt], asums[mt])
        if mt == 0:
            proj_t_pair(1, 0, evict_act=False)
        elif mt == 1:
            proj_t_pair(1, 1, evict_act=False)
        elif mt == 3:
            tr_x(2, cast_act=True)
        elif mt == 4:
            proj_t_pair(2, 0, evict_act=False)
        elif mt == 5:
            proj_t_pair(2, 1, evict_act=False)
    tr_x(3, cast_act=True)

    # --- P2: finish the split m-tiles (chunks 2-3 + epilogue), then the
    # remaining m-tiles in full; tT group 3 interleaves early in P2.
    for mt in range(NSPLIT):
        score_half(mt, 1, ets[mt], asums[mt])
        if mt == 0:
            # evicts stay on DVE: ACT's exps gate PSUM release here, and
            # loading ACT further measurably stalls the PE
            proj_t_pair(3, 0, evict_act=False)
        elif mt == 1:
            proj_t_pair(3, 1, evict_act=False)
        epilogue(mt, ets[mt], asums[mt])
    for mt in range(NSPLIT, ST):
        last_mt = mt == ST - 1
        if not last_mt:
            new_et_asum(mt)
            score_half(mt, 0, ets[mt], asums[mt])
            score_half(mt, 1, ets[mt], asums[mt])
            epilogue(mt, ets[mt], asums[mt])
        else:
            # last m-tile: fine-grained drain — 512-wide exp/normalize
            # chunks, DMAs alternating GpSimd/SP (SP last)
            et = epool.tile([P, SC, NCH], F32, tag="et", bufs=9)
            asum = spool.tile([P, SC + 1], F32, tag="asum", bufs=9)
            pss = [ps_tile(f"sl{i}") for i in range(2)]
            H = NCH // 2
            for ncn in range(SC):
                ps = pss[ncn // 2][:, ncn % 2, :]
                nsub = 1 if ncn < SC - 1 else 2
                for h2 in range(nsub):
                    w = NCH // nsub
                    for f2c in range(FT):
                        nc.tensor.matmul(
                            ps[:, h2 * w : (h2 + 1) * w],
                            tT[:, f2c, mt * P : (mt + 1) * P],
                            xT[:, f2c, ncn * NCH + h2 * w : ncn * NCH + (h2 + 1) * w],
                            start=(f2c == 0),
                            stop=(f2c == FT - 1),
                        )
                    nc.scalar.activation(
                        et[:, ncn, h2 * w : (h2 + 1) * w],
                        ps[:, h2 * w : (h2 + 1) * w],
                        Act.Exp,
                        scale=inv_sqrt_d,
                        accum_out=asum[:, ncn + h2 : ncn + h2 + 1],
                    )
            rsum = spool.tile([P, 1], F32, tag="rsum")
            nc.vector.reduce_sum(rsum[:], asum[:], axis=mybir.AxisListType.X)
            rrec = spool.tile([P, 1], F32, tag="rrec")
            nc.vector.reciprocal(rrec[:], rsum[:])
            ot = opool.tile([P, SC, NCH], BF16 if OUT_BF16 else F32)
            for q in range(SC - 1):
                sl = slice(q * NCH, (q + 1) * NCH)
                if q % 2 == 0:
                    nc.vector.tensor_scalar_mul(ot[:, q, :], et[:, q, :], rrec[:])
                else:
                    nc.scalar.activation(ot[:, q, :], et[:, q, :], Act.Identity, scale=rrec[:])
                dma_eng = nc.gpsimd if q % 2 == 0 else nc.sync
                dma_eng.dma_start(out_ext.ap()[mt * P : (mt + 1) * P, sl], ot[:, q, :])
            q = SC - 1
            for h2 in range(2):
                s2 = slice(h2 * H, (h2 + 1) * H)
                if h2 == 0:
                    nc.vector.tensor_scalar_mul(ot[:, q, s2], et[:, q, s2], rrec[:])
                else:
                    nc.scalar.activation(ot[:, q, s2], et[:, q, s2], Act.Identity, scale=rrec[:])
                dma_eng = nc.gpsimd if h2 == 0 else nc.sync
                dma_eng.dma_start(
                    out_ext.ap()[mt * P : (mt + 1) * P, q * NCH + h2 * H : q * NCH + (h2 + 1) * H],
                    ot[:, q, s2],
                )


_CACHE = {}


def build():
    if "nc" in _CACHE:
        return _CACHE["nc"]
    from contextlib import ExitStack

    nc = bacc.Bacc("TRN2", target_bir_lowering=False, debug=False, num_devices=B)
    x_ext = nc.dram_tensor("x", [S, F], F32, kind="ExternalInput")
    wq_ext = nc.dram_tensor("Wq", [F, D], F32, kind="ExternalInput")
    wk_ext = nc.dram_tensor("Wk", [F, D], F32, kind="ExternalInput")
    bq_ext = nc.dram_tensor("bq", [D], F32, kind="ExternalInput")
    out_ext = nc.dram_tensor(
        "out", [S, S], BF16 if OUT_BF16 else F32, kind="ExternalOutput"
    )

    with tile.TileContext(nc) as tc:
        with ExitStack() as ctx:
            _emit(nc, tc, ctx, x_ext, wq_ext, wk_ext, bq_ext, out_ext)

    nc.compile()
    _CACHE["nc"] = nc
    return nc


def make_in_maps(x, Wq, bq, Wk):
    x = np.ascontiguousarray(np.asarray(x, dtype=np.float32))
    Wq = np.ascontiguousarray(np.asarray(Wq, dtype=np.float32))
    Wk = np.ascontiguousarray(np.asarray(Wk, dtype=np.float32))
    bq = np.ascontiguousarray(np.asarray(bq, dtype=np.float32))
    return [{"x": x[i], "Wq": Wq, "Wk": Wk, "bq": bq} for i in range(B)]


def kernel(x, Wq, bq, Wk, bk=None, Wv=None, bv=None, **_unused):
    nc = build()
    in_maps = make_in_maps(x, Wq, bq, Wk)
    res = run_bass_kernel_spmd(nc, in_maps, core_ids=list(range(B)))
    return np.stack(
        [np.asarray(res.results[i]["out"], dtype=np.float32) for i in range(B)], axis=0
    )


# revision 43
# speedup vs baseline: 1.1736x; 1.0230x over previous
"""Trainium2 Bass kernel for nn_AttentionLayer: softmax(Q K^T / sqrt(d)).

Data-parallel over batch: 8 batch elements -> 8 NeuronCores, weights
replicated, no collectives.

Algebraic restructure (exact, softmax-invariant): with q = x Wq + bq and
k = x Wk + bk,
    q k^T = x (Wq Wk^T) x^T  +  1 (x Wk bq)^T  +  [terms constant along n]
and row-softmax drops any per-row constant, so
    alpha = softmax_n( (t x^T) / sqrt(d) ),   t = x W' + 1 c2^T,
    W' = Wq Wk^T  (512x512),  c2 = Wk bq.
This replaces one of the two [2048x512x512] projections with a single
[512x512x512] matmul (W') — ~6.6us less PE work per core — and removes
the bk load entirely.

Per core:
  xT    = transpose(x)            (PE f32 transposes, DVE evict->bf16)
  WqT/WkT = transpose(Wq/Wk)      (PE f32 transposes, ACT evict->bf16)
  W'    = WqT^T @ WkT             (TensorE bf16, ACT evict->bf16)
  c2    = WkT^T @ bq              (16 tiny N=1 matmuls, f32 PSUM accum)
  tT    = W'-chunks @ xT + c2     (TensorE bf16, bias evict ACT/DVE)
  S     = tT^T @ xT               (TensorE bf16, accumulate over f-tiles)
  E     = exp(S / sqrt(d)) with fused row-sum accumulate (ACT)
  out   = E / rowsum              (DVE per-partition scalar mul -> bf16)

Schedule notes (from NTFF traces): PE matmul throughput is at roofline
when dense (216ns per 512-wide bf16 MM). The input stream is the
startup constraint — ~6MB at the shared-HBM envelope takes ~25us — so
(a) the 2MB of weights loads FIRST, split over the SP/ACT/GpSimd
queues, because the serial W' -> tT chain depends on them, and (b) the
scores loop is split into half-tiles: each m-tile's n-chunks 0-1 touch
only x groups 0-1 and run while groups 2-3 are still streaming in; the
chunk 2-3 halves and the softmax epilogues follow once xg3 lands. This
keeps the PE dense from first data to last matmul. ACT stays exp-only
in the scores phase (output DMAs issue from SP and GpSimd-SWDGE,
normalization on DVE) so the epilogue keeps pace with the PE. The
end-of-kernel semaphore teardown (~12us for the framework-fixed 254
sems) and ~6us engine-init preamble are fixed costs. The DRAM output is
bf16 (halves the ~17MB/core output stream), upconverted to f32 on the
host; rel err vs the fp32 reference is ~4.5e-3.
"""

import os
import sys

sys.path.insert(0, "/opt/trn_rl_repo")

import numpy as np

import concourse.mybir as mybir
import concourse.tile as tile
from concourse import bacc
from concourse.bass_utils import run_bass_kernel_spmd
from concourse.masks import make_identity

B, S, F, D = 8, 2048, 512, 512
P = 128
ST = S // P   # 16 s-tiles
FT = F // P   # 4  f-tiles (contraction for projections / scores)
NCH = 512     # moving-operand / PSUM-bank chunk along the free axis
SC = S // NCH  # 4 chunks of the s axis
NSPLIT = 7    # m-tiles whose chunk-0/1 halves run ahead of xg3

F32 = mybir.dt.float32
BF16 = mybir.dt.bfloat16

# initial warmups sized to end just under Wq's ~13.5us arrival (they start
# ~7.6us now that the operand comes from a DVE memset); overshooting the
# arrival delays trw0 on the in-order PE queue and is measurably worse
WARMUP_MMS = int(os.environ.get("BASS_ATTN_WARMUP", "8"))
OUT_BF16 = os.environ.get("BASS_ATTN_OUT_BF16", "1") == "1"


def _emit(nc, tc, ctx, x_ext, wq_ext, wk_ext, bq_ext, out_ext):
    Act = mybir.ActivationFunctionType

    consts = ctx.enter_context(tc.tile_pool(name="consts", bufs=1))
    persist = ctx.enter_context(tc.tile_pool(name="persist", bufs=1))
    xstage = ctx.enter_context(tc.tile_pool(name="xstage", bufs=4))
    psum = ctx.enter_context(tc.tile_pool(name="psum", bufs=4, space="PSUM"))
    epool = ctx.enter_context(tc.tile_pool(name="epool", bufs=9))
    opool = ctx.enter_context(tc.tile_pool(name="opool", bufs=2))
    spool = ctx.enter_context(tc.tile_pool(name="spool", bufs=4))

    def ps_tile(name):
        # single unified PSUM tag: 4 bufs x [P, 2, 512] f32 = all 8 banks
        return psum.tile([P, 2, NCH], F32, tag="ps", bufs=4, name=name)

    ident = consts.tile([P, P], F32)
    make_identity(nc, ident[:])
    # Warmup operand from a DVE memset: available ~1.8us before gpsimd's
    # identity (gpsimd's engine preamble is the longest), so the PE starts
    # warming the HAM clock gate that much sooner.
    wrm = consts.tile([P, P], F32)
    nc.vector.memset(wrm[:], 0.0)

    def warm(n, name):
        # short (~0.2-0.4us) garbage matmuls: fill a known data-arrival or
        # evict-latency stall seam to keep the HAM clock-gate fed, sized
        # well under the seam so they never delay real work (an overshooting
        # warmup measurably pushes the whole weight chain back)
        wps = ps_tile(name)
        for _ in range(n):
            nc.tensor.matmul(wps[:, 0, :P], wrm[:], wrm[:], start=True, stop=True)

    # --- PE warmup: garbage matmuls while input DMAs land (HAM -> K=8/8)
    if WARMUP_MMS:
        warm(WARMUP_MMS, "warmps")

    from concourse.tile import add_dep_helper

    def gate(first_insts, prev_insts):
        for fi in first_insts:
            for pi in prev_insts:
                add_dep_helper(fi.ins, pi.ins, reason="input DMA phase chain")

    # --- input streaming.  Per-queue concurrency bounds throughput (a
    # single transfer moves ~25GB/s, one queue ~100-250GB/s), and the
    # whole 6MB runs at the shared-HBM envelope either way, so order by
    # NEED: the 2MB of weights first (they gate the serial W' -> tT g0
    # chain), split over all three queues; then the x groups as 64-row
    # half-tiles, rows 0-63 on the SP chain and rows 64-127 on GpSimd.
    wq_st = xstage.tile([P, FT, D], F32, tag="wstage", bufs=2, name="wqst")
    wk_st = xstage.tile([P, FT, D], F32, tag="wstage", bufs=2, name="wkst")

    def wsub(eng, wst, w_ext, ft):
        return eng.dma_start(wst[:, ft, :], w_ext.ap()[ft * P : (ft + 1) * P, :])

    # ACT queue: Wq halves + bq (ungated, from t=0)
    wsub(nc.scalar, wq_st, wq_ext, 0)
    wsub(nc.scalar, wq_st, wq_ext, 1)
    bqf = consts.tile([P, FT], F32)
    nc.scalar.dma_start(bqf[:], bq_ext.ap().rearrange("(dt p) -> p dt", p=P))
    # SP queue: rest of Wq + half of Wk, then the x chain
    sp_prev = [
        wsub(nc.sync, wq_st, wq_ext, 2),
        wsub(nc.sync, wq_st, wq_ext, 3),
        wsub(nc.sync, wk_st, wk_ext, 0),
        wsub(nc.sync, wk_st, wk_ext, 1),
    ]
    # GpSimd (SWDGE) queue: rest of Wk, then the x chain
    gp_prev = [
        wsub(nc.gpsimd, wk_st, wk_ext, 2),
        wsub(nc.gpsimd, wk_st, wk_ext, 3),
    ]

    def load_x_group_half(t, sg, eng, half):
        insts = []
        lo, hi = (0, 64) if half == 0 else (64, P)
        for j in range(4):
            st = sg * 4 + j
            insts.append(
                eng.dma_start(
                    t[lo:hi, j, :], x_ext.ap()[st * P + lo : st * P + hi, :]
                )
            )
        return insts

    # x groups phase-gated per queue so the earlier-needed groups get the
    # queues' full bandwidth
    xgroups = {}
    for sg in range(SC):
        xgroups[sg] = xstage.tile([P, 4, F], F32, tag="xstage", bufs=4, name=f"xg{sg}")
        sp_insts = load_x_group_half(xgroups[sg], sg, nc.sync, 0)
        gp_insts = load_x_group_half(xgroups[sg], sg, nc.gpsimd, 1)
        gate(sp_insts[:1], sp_prev)
        gate(gp_insts[:1], gp_prev)
        sp_prev, gp_prev = sp_insts, gp_insts

    # persistent bf16 operands
    xT = persist.tile([P, FT, S], BF16, name="xT")       # [f(part), ftile, s]
    wT = [persist.tile([P, FT, D], BF16, name=f"wT{w}") for w in range(2)]
    wp = persist.tile([P, FT, D], BF16, name="wp")       # W' [f1(part), f1t, f2]
    tT = persist.tile([P, FT, S], BF16, name="tT")       # [f2(part), f2t, m]
    c2 = consts.tile([P, FT], F32)                       # bias per f2 partition
    bqb = consts.tile([P, FT], BF16)

    def tr_x(sg, cast_act=False):
        # xT[ft][p, s] = x[s, ft*128+p] for this s-group; two f-tiles per
        # 2-bank PSUM tile, one merged [P,2,512] eviction each.  (An
        # f32->bf16 pre-cast + bf16 transposes was tried and is SLOWER
        # end-to-end: the cast adds ~2us latency on each group's critical
        # path, which outweighs the halved PE transpose cost.)
        xts = xgroups[sg]
        for fp in range(2):
            ps = ps_tile(f"tr{sg}{fp}")
            for k in range(2):
                ft = 2 * fp + k
                for j in range(4):
                    nc.tensor.transpose(
                        ps[:, k, j * P : (j + 1) * P],
                        xts[:, j, ft * P : (ft + 1) * P],
                        ident[:],
                    )
            nc.vector.tensor_copy(
                xT[:, 2 * fp : 2 * fp + 2, sg * NCH : (sg + 1) * NCH], ps[:]
            )

    def tr_w(w, wst):
        # wT[w][p, dt, f] = W[f, dt*128+p]
        for dp in range(2):
            ps = ps_tile(f"wtr{w}{dp}")
            for k in range(2):
                dt = 2 * dp + k
                for ft in range(FT):
                    nc.tensor.transpose(
                        ps[:, k, ft * P : (ft + 1) * P],
                        wst[:, ft, dt * P : (dt + 1) * P],
                        ident[:],
                    )
            nc.scalar.activation(wT[w][:, 2 * dp : 2 * dp + 2, :], ps[:], Act.Identity)

    def emit_wprime():
        # W'[f1, f2] = sum_d Wq[f1, d] Wk[f2, d] = WqT^T @ WkT
        for fp in range(2):
            ps = ps_tile(f"wp{fp}")
            for k in range(2):
                f1c = 2 * fp + k
                for dt in range(FT):
                    nc.tensor.matmul(
                        ps[:, k, :],
                        wT[0][:, dt, f1c * P : (f1c + 1) * P],
                        wT[1][:, dt, :],
                        start=(dt == 0),
                        stop=(dt == FT - 1),
                    )
            nc.scalar.activation(wp[:, 2 * fp : 2 * fp + 2, :], ps[:], Act.Identity)
        # c2[f2] = sum_d Wk[f2, d] bq[d]; tiny N=1 matmuls, f32 PSUM accum
        nc.vector.tensor_copy(bqb[:], bqf[:])
        cps = ps_tile("c2ps")
        for f2c in range(FT):
            for dt in range(FT):
                nc.tensor.matmul(
                    cps[:, 0, f2c : f2c + 1],
                    wT[1][:, dt, f2c * P : (f2c + 1) * P],
                    bqb[:, dt : dt + 1],
                    start=(dt == 0),
                    stop=(dt == FT - 1),
                )
        nc.vector.tensor_copy(c2[:], cps[:, 0, :FT])

    def proj_t_pair(mg, fp, evict_act=True):
        # tT[f2, m] = sum_f1 W'[f1, f2] xT[f1, m] + c2[f2] for f2 chunk pair
        ps = ps_tile(f"pj{mg}{fp}")
        for k in range(2):
            f2c = 2 * fp + k
            for f1c in range(FT):
                nc.tensor.matmul(
                    ps[:, k, :],
                    wp[:, f1c, f2c * P : (f2c + 1) * P],
                    xT[:, f1c, mg * NCH : (mg + 1) * NCH],
                    start=(f1c == 0),
                    stop=(f1c == FT - 1),
                )
        for k in range(2):
            f2c = 2 * fp + k
            dst = tT[:, f2c, mg * NCH : (mg + 1) * NCH]
            bias = c2[:, f2c : f2c + 1]
            if evict_act:
                nc.scalar.activation(dst, ps[:, k, :], Act.Identity, bias=bias)
            else:
                nc.vector.tensor_scalar_add(dst, ps[:, k, :], bias)

    inv_sqrt_d = 1.0 / float(np.sqrt(np.float32(D)))
    ets = {}
    asums = {}

    def score_half(mt, h, et, asum):
        # chunks 2h, 2h+1 of m-tile mt: 8 MMs + one fused exp/accumulate
        ps = ps_tile(f"s{mt}_{h}")
        for k in range(2):
            ncn = 2 * h + k
            for f2c in range(FT):
                nc.tensor.matmul(
                    ps[:, k, :],
                    tT[:, f2c, mt * P : (mt + 1) * P],
                    xT[:, f2c, ncn * NCH : (ncn + 1) * NCH],
                    start=(f2c == 0),
                    stop=(f2c == FT - 1),
                )
        nc.scalar.activation(
            et[:, 2 * h : 2 * h + 2, :],
            ps[:],
            Act.Exp,
            scale=inv_sqrt_d,
            accum_out=asum[:, h : h + 1],
        )

    def epilogue(mt, et, asum):
        rsum = spool.tile([P, 1], F32, tag="rsum")
        nc.vector.reduce_sum(rsum[:], asum[:], axis=mybir.AxisListType.X)
        rrec = spool.tile([P, 1], F32, tag="rrec")
        nc.vector.reciprocal(rrec[:], rsum[:])
        ot = opool.tile([P, SC, NCH], BF16 if OUT_BF16 else F32)
        for h in range(2):
            nc.vector.tensor_scalar_mul(
                ot[:, 2 * h : 2 * h + 2, :], et[:, 2 * h : 2 * h + 2, :], rrec[:]
            )
        # ONE merged output DMA per m-tile, SP/GpSimd alternating; ACT
        # stays exp-only so the epilogue keeps pace with the PE
        dma_eng = nc.sync if mt % 2 == 0 else nc.gpsimd
        dma_eng.dma_start(out_ext.ap()[mt * P : (mt + 1) * P, :], ot[:])

    def new_et_asum(mt):
        ets[mt] = epool.tile([P, SC, NCH], F32, tag="et", bufs=9, name="et")
        asums[mt] = spool.tile([P, 2], F32, tag="asum", bufs=9, name="asum")

    # --- pre-scores: weight-derived chain first (weights land first);
    # short warmup fills in the measured stall seams (Wk arrival ~3.4us
    # after Wq; ACT evict latency between transpose chain stages)
    tr_w(0, wq_st)
    warm(6, "warmw1")
    tr_w(1, wk_st)
    warm(2, "warmw2")
    emit_wprime()
    warm(2, "warmw3")
    tr_x(0)
    proj_t_pair(0, 0)
    proj_t_pair(0, 1)
    tr_x(1)

    # --- P1: chunk-0/1 halves of the first NSPLIT m-tiles (need only
    # x groups 0-1) run while x groups 2-3 are still streaming in;
    # deferred tT projections for m-groups 1-2 interleave (their moving
    # operand is x group 1 / 2 respectively).
    for mt in range(NSPLIT):
        new_et_asum(mt)
        score_half(mt, 0, ets[mt], asums[mt])
        if mt == 0:
            proj_t_pair(1, 0, evict_act=False)
        elif mt == 1:
            proj_t_pair(1, 1, evict_act=False)
        elif mt == 3:
            tr_x(2, cast_act=True)
        elif mt == 4:
            proj_t_pair(2, 0, evict_act=False)
        elif mt == 5:
            proj_t_pair(2, 1, evict_act=False)
    tr_x(3, cast_act=True)

    # --- P2: finish the split m-tiles (chunks 2-3 + epilogue), then the
    # remaining m-tiles in full; tT group 3 interleaves early in P2.
    for mt in range(NSPLIT):
        score_half(mt, 1, ets[mt], asums[mt])
        if mt == 0:
            # evicts stay on DVE: ACT's exps gate PSUM release here, and
            # loading ACT further measurably stalls the PE
            proj_t_pair(3, 0, evict_act=False)
        elif mt == 1:
            proj_t_pair(3, 1, evict_act=False)
        epilogue(mt, ets[mt], asums[mt])
    for mt in range(NSPLIT, ST):
        last_mt = mt == ST - 1
        if not last_mt:
            new_et_asum(mt)
            score_half(mt, 0, ets[mt], asums[mt])
            score_half(mt, 1, ets[mt], asums[mt])
            epilogue(mt, ets[mt], asums[mt])
        else:
            # last m-tile: fine-grained drain — 512-wide exp/normalize
            # chunks, DMAs alternating GpSimd/SP (SP last)
            et = epool.tile([P, SC, NCH], F32, tag="et", bufs=9)
            asum = spool.tile([P, SC], F32, tag="asum", bufs=9)
            pss = [ps_tile(f"sl{i}") for i in range(2)]
            for ncn in range(SC):
                ps = pss[ncn // 2][:, ncn % 2, :]
                for f2c in range(FT):
                    nc.tensor.matmul(
                        ps,
                        tT[:, f2c, mt * P : (mt + 1) * P],
                        xT[:, f2c, ncn * NCH : (ncn + 1) * NCH],
                        start=(f2c == 0),
                        stop=(f2c == FT - 1),
                    )
                nc.scalar.activation(
                    et[:, ncn, :],
                    ps,
                    Act.Exp,
                    scale=inv_sqrt_d,
                    accum_out=asum[:, ncn : ncn + 1],
                )
            rsum = spool.tile([P, 1], F32, tag="rsum")
            nc.vector.reduce_sum(rsum[:], asum[:], axis=mybir.AxisListType.X)
            rrec = spool.tile([P, 1], F32, tag="rrec")
            nc.vector.reciprocal(rrec[:], rsum[:])
            ot = opool.tile([P, SC, NCH], BF16 if OUT_BF16 else F32)
            for q in range(SC):
                sl = slice(q * NCH, (q + 1) * NCH)
                if q % 2 == 0:
                    nc.vector.tensor_scalar_mul(ot[:, q, :], et[:, q, :], rrec[:])
                else:
                    nc.scalar.activation(ot[:, q, :], et[:, q, :], Act.Identity, scale=rrec[:])
                dma_eng = nc.gpsimd if q % 2 == 0 else nc.sync
                dma_eng.dma_start(out_ext.ap()[mt * P : (mt + 1) * P, sl], ot[:, q, :])


_CACHE = {}


def build():
    if "nc" in _CACHE:
        return _CACHE["nc"]
    from contextlib import ExitStack

    nc = bacc.Bacc("TRN2", target_bir_lowering=False, debug=False, num_devices=B)
    x_ext = nc.dram_tensor("x", [S, F], F32, kind="ExternalInput")
    wq_ext = nc.dram_tensor("Wq", [F, D], F32, kind="ExternalInput")
    wk_ext = nc.dram_tensor("Wk", [F, D], F32, kind="ExternalInput")
    bq_ext = nc.dram_tensor("bq", [D], F32, kind="ExternalInput")
    out_ext = nc.dram_tensor(
        "out", [S, S], BF16 if OUT_BF16 else F32, kind="ExternalOutput"
    )

    with tile.TileContext(nc) as tc:
        with ExitStack() as ctx:
            _emit(nc, tc, ctx, x_ext, wq_ext, wk_ext, bq_ext, out_ext)

    nc.compile()
    _CACHE["nc"] = nc
    return nc


def make_in_maps(x, Wq, bq, Wk):
    x = np.ascontiguousarray(np.asarray(x, dtype=np.float32))
    Wq = np.ascontiguousarray(np.asarray(Wq, dtype=np.float32))
    Wk = np.ascontiguousarray(np.asarray(Wk, dtype=np.float32))
    bq = np.ascontiguousarray(np.asarray(bq, dtype=np.float32))
    return [{"x": x[i], "Wq": Wq, "Wk": Wk, "bq": bq} for i in range(B)]


def kernel(x, Wq, bq, Wk, bk=None, Wv=None, bv=None, **_unused):
    nc = build()
    in_maps = make_in_maps(x, Wq, bq, Wk)
    res = run_bass_kernel_spmd(nc, in_maps, core_ids=list(range(B)))
    return np.stack(
        [np.asarray(res.results[i]["out"], dtype=np.float32) for i in range(B)], axis=0
    )
